# revision 20
# baseline (speedup 1.0000x reference)
"""Causal self-attention Trainium2 kernel (8-core head-parallel tensor parallel).

v3 — all-bf16 I/O dataflow, fine-grained cross-chunk weaving.

Strategy (per core, 2 heads):
  - Host: x^T bf16 (shared), per-core W_qkv slice^T bf16, W_proj col-slice^T
    bf16, qkv bias slice f32.
  - Device, per (batch b, 512-token chunk): attention chains
      S^T tile = K_tile^T.T @ Q^T (PE, f32 PSUM, diagonal-trimmed columns)
      e = exp(0.125*S)            (ACT bf16; ACT does nothing else steady-state)
      causal triangle zeroing     (GPSIMD affine_select, off the PE/ACT path)
      psy += [V|1]^T.T @ e        (PE accumulate; row 64 = Z for free)
      1/Z (DVE) -> PE rank-1 broadcast -> ACT copy -> DVE mul -> y^T bf16
    are WOVEN, slot by slot, with ACT-independent PE filler work: the next
    chunk's QKV matmul chains + V^T transposes and the previous chunk's
    projection matmuls. Each AV trails its S by DEPTH=3 attention slots plus
    the filler in between (~2+ us of wall time), so the PE instruction stream
    never blocks on the scalar engine's exp. This keeps the PE HAM clock gate
    at 2.4 GHz; earlier versions without weaving ran matmuls at half clock
    65-78% of the time.
  - out^T partial (bf16) summed + b_proj on host.
"""

import sys

if "/opt/trn_rl_repo" not in sys.path:
    sys.path.insert(0, "/opt/trn_rl_repo")

from collections import deque

import numpy as np

# ---- problem constants (hardcoded for the grading harness) ----
B, T, C, H = 2, 2048, 1024, 16
HD = C // H            # 64
N_CORES = 8
HPC = H // N_CORES     # heads per core = 2


def _cfg_full():
    return dict(B=B, T=T, C=C, HPC=HPC)


def build_nc(cfg):
    """Build the single-core SPMD Bass program."""
    import concourse.bacc as bacc
    import concourse.mybir as mybir
    import concourse.tile as tile
    from concourse.masks import make_identity

    Bc, Tc, Cc, hpc = cfg["B"], cfg["T"], cfg["C"], cfg["HPC"]
    f32 = mybir.dt.float32
    f32r = mybir.dt.float32r
    bf16 = mybir.dt.bfloat16
    BT = Bc * Tc
    MQ = hpc * HD                 # rows per m-group (q|k|v) = 128
    assert MQ == 128
    KT_C = Cc // 128              # contraction tiles for qkv/x = 8
    TOKC = 512
    QC = Tc // TOKC               # q-chunks per batch = 4
    KPC = TOKC // 128             # k-tiles per chunk = 4
    MO = Cc // 128                # proj output tiles = 8
    NCH = BT // TOKC              # total chunks = 8
    DEPTH = 3                     # S->AV pipeline depth in attention slots

    AF = mybir.ActivationFunctionType
    ALU = mybir.AluOpType

    nc = bacc.Bacc()
    xT = nc.declare_dram_parameter("xT", [Cc, BT], bf16, isOutput=False)
    wqkvT = nc.declare_dram_parameter("wqkvT", [Cc, 3 * MQ], bf16, isOutput=False)
    bqkv = nc.declare_dram_parameter("bqkv", [3 * MQ, 1], f32, isOutput=False)
    wpT = nc.declare_dram_parameter("wpT", [MQ, Cc], bf16, isOutput=False)
    outT = nc.declare_dram_parameter("outT", [Cc, BT], bf16, isOutput=True)

    xT_r = xT.rearrange("(kt p) t -> p kt t", p=128)
    wq_r = wqkvT.rearrange("(kt p) m -> p kt m", p=128)
    bq_r = bqkv.rearrange("(g p) o -> p (g o)", p=128)

    with tile.TileContext(nc) as tc:
        with (
            tc.tile_pool(name="consts", bufs=1) as consts,
            tc.tile_pool(name="spool", bufs=6) as spool,
            tc.tile_pool(name="ypool", bufs=2) as ypool,
            tc.tile_pool(name="npool", bufs=2) as npool,
            tc.tile_pool(name="nbpool", bufs=2) as nbpool,
            tc.tile_pool(name="opool", bufs=3) as opool,
            tc.tile_pool(name="ps_s", bufs=4, space="PSUM") as ps_s,
            tc.tile_pool(name="ps_mm", bufs=2, space="PSUM") as ps_mm,
            tc.tile_pool(name="ps_y", bufs=2, space="PSUM") as ps_y,
        ):
            # ---- constants / upfront DMAs (x chunk 0 first: startup path) ----
            x_sb = consts.tile([128, KT_C, BT], bf16, tag="x")
            w_sb = consts.tile([128, KT_C, 3 * MQ], bf16, tag="w")
            # staged pieces: the first QKV matmuls wait only on their own
            # kt slices, and dispatch overhead stays low (6 DMAs, not 16)
            for ksl in (slice(0, 1), slice(1, 2), slice(2, KT_C)):
                nc.sync.dma_start(out=w_sb[:, ksl, :], in_=wq_r[:, ksl, :])
                nc.sync.dma_start(out=x_sb[:, ksl, 0:TOKC],
                                  in_=xT_r[:, ksl, 0:TOKC])
            b_sb = consts.tile([128, 3], f32, tag="b")
            nc.sync.dma_start(out=b_sb, in_=bq_r)
            for ch in range(1, NCH):
                sl = slice(ch * TOKC, (ch + 1) * TOKC)
                nc.sync.dma_start(out=x_sb[:, :, sl], in_=xT_r[:, :, sl])
            wp_sb = consts.tile([128, Cc], bf16, tag="wp")
            nc.sync.dma_start(out=wp_sb, in_=wpT[:, :])

            ident = consts.tile([128, 128], f32, tag="id")
            make_identity(nc, ident)
            ident_bf = consts.tile([128, 128], bf16, tag="idb")
            nc.vector.tensor_copy(ident_bf[:, :], ident[:, :])
            ones_f32 = consts.tile([1, HD], f32, tag="ones_f")
            nc.vector.memset(ones_f32[:, :], 1.0)
            ones_sb = consts.tile([1, HD], f32r, tag="ones")
            nc.scalar.activation(out=ones_sb[:, :], in_=ones_f32[:, :],
                                 func=AF.Copy)

            # qkv^T feature-major [128, BT] bf16
            qT = consts.tile([128, BT], bf16, tag="qT")
            kT = consts.tile([128, BT], bf16, tag="kT")
            vT = consts.tile([128, BT], bf16, tag="vT")
            # V in [token-part, kt, head, d|1] layout; col 64 = 1.0 (Z row)
            v_sb = [
                consts.tile([128, QC * KPC, hpc, 65], bf16, tag=f"v{b}",
                            name=f"v{b}") for b in range(Bc)
            ]
            for b in range(Bc):
                nc.gpsimd.memset(v_sb[b][:, :, :, 64:65], 1.0)

            # ------------- emission building blocks -------------
            def qkv_steps(ch):
                """Filler closures: 24 QKV matmuls (+1 eviction each chain)."""
                t_sl = slice(ch * TOKC, (ch + 1) * TOKC)
                steps = []
                for m in range(3):
                    ps_ref = {}
                    for kt in range(KT_C):
                        def step(m=m, kt=kt, ps_ref=ps_ref):
                            if kt == 0:
                                ps_ref[0] = ps_mm.tile([128, TOKC], f32,
                                                       tag="mm", name="qkvps")
                            nc.tensor.matmul(
                                ps_ref[0][:, :],
                                w_sb[:, kt, m * MQ:(m + 1) * MQ],
                                x_sb[:, kt, t_sl],
                                start=(kt == 0), stop=(kt == KT_C - 1))
                            if kt == KT_C - 1:
                                dst = (qT, kT, vT)[m]
                                nc.vector.tensor_scalar_add(
                                    dst[:, t_sl], ps_ref[0][:, :],
                                    b_sb[:, m:m + 1])
                        steps.append(step)
                return steps

            def vt_steps(ch):
                """Filler closures: 4 V^T transposes (+2 DVE evictions each)."""
                b, qc = ch // QC, ch % QC
                steps = []
                for i in range(KPC):
                    kt = qc * KPC + i

                    def step(b=b, kt=kt):
                        pst = ps_s.tile([128, 128], bf16, tag="s", name="vtps")
                        nc.tensor.transpose(
                            pst[:, :],
                            vT[:, b * Tc + kt * 128:b * Tc + (kt + 1) * 128],
                            ident_bf[:, :])
                        for hh in range(hpc):
                            nc.vector.tensor_copy(
                                v_sb[b][:, kt, hh, 0:HD],
                                pst[:, hh * HD:(hh + 1) * HD])
                    steps.append(step)
                return steps

            def proj_steps(b, qc, yT_t, evict_act=False):
                """Filler closures: 8 projection matmuls + evict + DMA.
                evict_act routes the eviction to the scalar engine — used for
                the final chunks where ACT is otherwise idle and the DVE
                backlog was stalling the drain."""
                q_sl = slice(b * Tc + qc * TOKC, b * Tc + (qc + 1) * TOKC)
                steps = []
                for mo in range(MO):
                    def step(mo=mo):
                        pso = ps_mm.tile([128, TOKC], f32, tag="mm",
                                         name="projps")
                        nc.tensor.matmul(
                            pso[:, :], wp_sb[:, mo * 128:(mo + 1) * 128],
                            yT_t[:, :], start=True, stop=True)
                        o_t = opool.tile([128, TOKC], bf16, tag="o")
                        if evict_act and mo % 2 == 0:
                            nc.scalar.activation(out=o_t[:, :], in_=pso[:, :],
                                                 func=AF.Copy)
                        else:
                            nc.vector.tensor_copy(o_t[:, :], pso[:, :])
                        nc.sync.dma_start(
                            out=outT[mo * 128:(mo + 1) * 128, q_sl],
                            in_=o_t[:, :])
                    steps.append(step)
                return steps

            def emit_norm(hh, yT_t, psy, zrow):
                """y^T[hh] = psy[0:64] / broadcast(Z): PE rank-1 broadcast of
                the Z row, then a fast DVE reciprocal on the broadcast (the
                exact DVE `reciprocal` is ~6.5 ns/elem and blocked the DVE
                FIFO for 3.4 us at every chain end)."""
                ps_b = ps_mm.tile([HD, TOKC], f32, tag="mm", name="bcps")
                nc.tensor.matmul(ps_b[:, :], ones_sb[:, :], zrow[:, :],
                                 start=True, stop=True)
                rcb = nbpool.tile([HD, TOKC], f32, tag="rcb")
                nc.vector.reciprocal_approx_fast(rcb[:, :], ps_b[:, :])
                nc.vector.tensor_mul(
                    yT_t[hh * HD:(hh + 1) * HD, :], psy[0:HD, :], rcb[:, :])

            # ------------- woven chunk emission -------------
            pend_av = deque()     # pending AV closures
            norm_pend = deque()   # [countdown_in_av_pops, closure]

            def pop_av():
                pend_av.popleft()()
                for item in norm_pend:
                    item[0] -= 1
                while norm_pend and norm_pend[0][0] <= 0:
                    norm_pend.popleft()[1]()

            def chunk_stream(b, qc, yT_t, filler, filler_late=False):
                n_kt = (qc + 1) * KPC
                qbase = b * Tc + qc * TOKC
                psys = {}

                def emit_S(hh, kt):
                    di = kt - qc * KPC
                    c0 = di * 128 if di >= 0 else 0
                    pss = ps_s.tile([128, TOKC], f32, tag="s")
                    nc.tensor.matmul(
                        pss[:, c0:TOKC],
                        kT[hh * HD:(hh + 1) * HD,
                           b * Tc + kt * 128:b * Tc + (kt + 1) * 128],
                        qT[hh * HD:(hh + 1) * HD, qbase + c0:qbase + TOKC],
                        start=True, stop=True)
                    e_t = spool.tile([128, TOKC], bf16, tag="e")
                    nc.scalar.activation(out=e_t[:, c0:TOKC], in_=pss[:, c0:TOKC],
                                         func=AF.Exp, scale=0.125)
                    if di >= 0:
                        # zero strict upper triangle of the diagonal 128x128
                        # block: keep iff (q - k) = f - p >= 0
                        nc.gpsimd.affine_select(
                            out=e_t[:, c0:c0 + 128], in_=e_t[:, c0:c0 + 128],
                            compare_op=ALU.is_ge, fill=0.0, base=0,
                            pattern=[[1, 128]], channel_multiplier=-1)
                    return e_t, c0

                def mk_av(hh, kt, e_t, c0, psy):
                    def emit():
                        nc.tensor.matmul(
                            psy[:, c0:TOKC], v_sb[b][:, kt, hh, :],
                            e_t[:, c0:TOKC],
                            start=(kt == 0), stop=(kt == n_kt - 1))
                        if kt == n_kt - 1:
                            zrow = npool.tile([1, TOKC], f32r, tag="rc")
                            nc.scalar.activation(out=zrow[:, :],
                                                 in_=psy[64:65, :], func=AF.Copy)
                            norm_pend.append(
                                [2, lambda: emit_norm(hh, yT_t, psy, zrow)])
                    return emit

                n_slots = hpc * n_kt
                slot = 0
                for kt in range(n_kt):
                    for hh in range(hpc):
                        if hh not in psys:
                            psys[hh] = ps_y.tile([65, TOKC], f32, tag="y",
                                                 name=f"psy{hh}")
                        e_t, c0 = emit_S(hh, kt)
                        pend_av.append(mk_av(hh, kt, e_t, c0, psys[hh]))
                        # pace the filler: keep some in reserve for late slots
                        # so tail AVs stay cushioned from their exps; placed
                        # between S and AV so the AV weight-load can prefetch
                        # behind the filler matmul
                        if filler and (len(filler) >= n_slots - slot
                                       or (slot % 2 == 0 and not filler_late)):
                            filler.popleft()()
                        if len(pend_av) > DEPTH:
                            pop_av()
                        slot += 1
                # drain: alternate remaining filler with pending AVs/norms
                while filler or pend_av or norm_pend:
                    if filler:
                        filler.popleft()()
                    if pend_av:
                        pop_av()
                    elif not filler:
                        while norm_pend:
                            norm_pend.popleft()[1]()

            # ---------------- main schedule ----------------
            # prologue: QKV + V^T for chunk 0, unwoven
            for step in qkv_steps(0):
                step()
            for step in vt_steps(0):
                step()

            pending_proj = None   # (b, qc, yT_t) of previous chunk
            for g in range(NCH):
                b, qc = g // QC, g % QC
                filler = deque()
                if g + 1 < NCH:
                    filler.extend(qkv_steps(g + 1))
                    filler.extend(vt_steps(g + 1))
                if pending_proj is not None:
                    filler.extend(proj_steps(*pending_proj,
                                             evict_act=(g == NCH - 1)))
                yT_t = ypool.tile([128, TOKC], bf16, tag="yT")
                chunk_stream(b, qc, yT_t, filler, filler_late=(g == NCH - 1))
                pending_proj = (b, qc, yT_t)
            # epilogue: projection of the last chunk
            for step in proj_steps(*pending_proj, evict_act=True):
                step()

    nc.finalize()
    return nc


def prep_inputs(cfg, x, W_attn, b_attn, W_proj, b_proj):
    """Host-side sharding: returns per-core input dicts."""
    import ml_dtypes

    Bc, Tc, Cc, hpc = cfg["B"], cfg["T"], cfg["C"], cfg["HPC"]
    n_cores = (Cc // HD) // hpc
    BT = Bc * Tc
    MQ = hpc * HD
    bf16 = ml_dtypes.bfloat16

    x = np.asarray(x, dtype=np.float32)
    xT = np.ascontiguousarray(x.reshape(BT, Cc).T.astype(bf16))

    in_maps = []
    for c in range(n_cores):
        r0 = c * MQ
        rows = []
        for g in range(3):
            rows.append(np.arange(g * Cc + r0, g * Cc + r0 + MQ))
        rows = np.concatenate(rows)
        w_slice = np.asarray(W_attn, dtype=np.float32)[rows, :]   # [384, C]
        wqkvT = np.ascontiguousarray(w_slice.T.astype(bf16))      # [C, 384]
        bq = np.ascontiguousarray(
            np.asarray(b_attn, dtype=np.float32)[rows].reshape(MQ * 3, 1))
        wpT = np.ascontiguousarray(
            np.asarray(W_proj, dtype=np.float32)[:, r0:r0 + MQ].T.astype(bf16))
        in_maps.append({
            "xT": xT,
            "wqkvT": wqkvT,
            "bqkv": bq.astype(np.float32),
            "wpT": wpT,
        })
    return in_maps


def combine(cfg, results, b_proj):
    Bc, Tc, Cc = cfg["B"], cfg["T"], cfg["C"]
    acc = results[0]["outT"].astype(np.float32)
    for r in results[1:]:
        acc = acc + r["outT"].astype(np.float32)
    out = acc.T + np.asarray(b_proj, dtype=np.float32)[None, :]
    return np.ascontiguousarray(out.reshape(Bc, Tc, Cc).astype(np.float32))


_NC_CACHE = {}


def kernel(x, W_attn, b_attn, W_proj, b_proj):
    from concourse.bass_utils import run_bass_kernel_spmd

    cfg = _cfg_full()
    key = "full"
    if key not in _NC_CACHE:
        _NC_CACHE[key] = build_nc(cfg)
    nc = _NC_CACHE[key]
    in_maps = prep_inputs(cfg, np.asarray(x), np.asarray(W_attn),
                          np.asarray(b_attn), np.asarray(W_proj),
                          np.asarray(b_proj))
    res = run_bass_kernel_spmd(nc, in_maps, list(range(N_CORES)))
    return combine(cfg, res.results, np.asarray(b_proj, dtype=np.float32))


# revision 21
# speedup vs baseline: 1.0713x; 1.0713x over previous
"""Causal self-attention Trainium2 kernel (8-core head-parallel tensor parallel).

v3 — all-bf16 I/O dataflow, fine-grained cross-chunk weaving.

Strategy (per core, 2 heads):
  - Host: x^T bf16 (shared), per-core W_qkv slice^T bf16, W_proj col-slice^T
    bf16, qkv bias slice f32.
  - Device, per (batch b, 512-token chunk): attention chains
      S^T tile = K_tile^T.T @ Q^T (PE, f32 PSUM, diagonal-trimmed columns)
      e = exp(0.125*S)            (ACT bf16; ACT does nothing else steady-state)
      causal triangle zeroing     (GPSIMD affine_select, off the PE/ACT path)
      psy += [V|1]^T.T @ e        (PE accumulate; row 64 = Z for free)
      1/Z (DVE) -> PE rank-1 broadcast -> ACT copy -> DVE mul -> y^T bf16
    are WOVEN, slot by slot, with ACT-independent PE filler work: the next
    chunk's QKV matmul chains + V^T transposes and the previous chunk's
    projection matmuls. Each AV trails its S by DEPTH=3 attention slots plus
    the filler in between (~2+ us of wall time), so the PE instruction stream
    never blocks on the scalar engine's exp. This keeps the PE HAM clock gate
    at 2.4 GHz; earlier versions without weaving ran matmuls at half clock
    65-78% of the time.
  - out^T partial (bf16) summed + b_proj on host.
"""

import sys

if "/opt/trn_rl_repo" not in sys.path:
    sys.path.insert(0, "/opt/trn_rl_repo")

from collections import deque

import numpy as np

# ---- problem constants (hardcoded for the grading harness) ----
B, T, C, H = 2, 2048, 1024, 16
HD = C // H            # 64
N_CORES = 8
HPC = H // N_CORES     # heads per core = 2


def _cfg_full():
    return dict(B=B, T=T, C=C, HPC=HPC)


def build_nc(cfg):
    """Build the single-core SPMD Bass program."""
    import concourse.bacc as bacc
    import concourse.mybir as mybir
    import concourse.tile as tile
    from concourse.masks import make_identity

    Bc, Tc, Cc, hpc = cfg["B"], cfg["T"], cfg["C"], cfg["HPC"]
    f32 = mybir.dt.float32
    f32r = mybir.dt.float32r
    bf16 = mybir.dt.bfloat16
    BT = Bc * Tc
    MQ = hpc * HD                 # rows per m-group (q|k|v) = 128
    assert MQ == 128
    KT_C = Cc // 128              # contraction tiles for qkv/x = 8
    TOKC = 512
    QC = Tc // TOKC               # q-chunks per batch = 4
    KPC = TOKC // 128             # k-tiles per chunk = 4
    MO = Cc // 128                # proj output tiles = 8
    NCH = BT // TOKC              # total chunks = 8
    DEPTH = 3                     # S->AV pipeline depth in attention slots

    AF = mybir.ActivationFunctionType
    ALU = mybir.AluOpType

    nc = bacc.Bacc()
    xT = nc.declare_dram_parameter("xT", [Cc, BT], bf16, isOutput=False)
    wqkvT = nc.declare_dram_parameter("wqkvT", [Cc, 3 * MQ], bf16, isOutput=False)
    bqkv = nc.declare_dram_parameter("bqkv", [3 * MQ, 1], f32, isOutput=False)
    wpT = nc.declare_dram_parameter("wpT", [MQ, Cc], bf16, isOutput=False)
    outT = nc.declare_dram_parameter("outT", [Cc, BT], bf16, isOutput=True)

    xT_r = xT.rearrange("(kt p) t -> p kt t", p=128)
    wq_r = wqkvT.rearrange("(kt p) m -> p kt m", p=128)
    bq_r = bqkv.rearrange("(g p) o -> p (g o)", p=128)

    with tile.TileContext(nc) as tc:
        with (
            tc.tile_pool(name="consts", bufs=1) as consts,
            tc.tile_pool(name="spool", bufs=6) as spool,
            tc.tile_pool(name="ypool", bufs=2) as ypool,
            tc.tile_pool(name="npool", bufs=2) as npool,
            tc.tile_pool(name="nbpool", bufs=2) as nbpool,
            tc.tile_pool(name="opool", bufs=3) as opool,
            tc.tile_pool(name="ps_s", bufs=4, space="PSUM") as ps_s,
            tc.tile_pool(name="ps_mm", bufs=2, space="PSUM") as ps_mm,
            tc.tile_pool(name="ps_y", bufs=2, space="PSUM") as ps_y,
        ):
            # ---- constants / upfront DMAs (x chunk 0 first: startup path) ----
            x_sb = consts.tile([128, KT_C, BT], bf16, tag="x")
            w_sb = consts.tile([128, KT_C, 3 * MQ], bf16, tag="w")
            # staged pieces: the first QKV matmuls wait only on their own
            # kt slices, and dispatch overhead stays low (6 DMAs, not 16)
            for ksl in (slice(0, 1), slice(1, 2), slice(2, KT_C)):
                nc.sync.dma_start(out=w_sb[:, ksl, :], in_=wq_r[:, ksl, :])
                nc.sync.dma_start(out=x_sb[:, ksl, 0:TOKC],
                                  in_=xT_r[:, ksl, 0:TOKC])
            b_sb = consts.tile([128, 3], f32, tag="b")
            nc.sync.dma_start(out=b_sb, in_=bq_r)
            for ch in range(1, NCH):
                sl = slice(ch * TOKC, (ch + 1) * TOKC)
                nc.sync.dma_start(out=x_sb[:, :, sl], in_=xT_r[:, :, sl])
            wp_sb = consts.tile([128, Cc], bf16, tag="wp")
            nc.sync.dma_start(out=wp_sb, in_=wpT[:, :])

            ident = consts.tile([128, 128], f32, tag="id")
            make_identity(nc, ident)
            ident_bf = consts.tile([128, 128], bf16, tag="idb")
            nc.vector.tensor_copy(ident_bf[:, :], ident[:, :])
            ones_f32 = consts.tile([1, HD], f32, tag="ones_f")
            nc.vector.memset(ones_f32[:, :], 1.0)
            ones_sb = consts.tile([1, HD], f32r, tag="ones")
            nc.scalar.activation(out=ones_sb[:, :], in_=ones_f32[:, :],
                                 func=AF.Copy)

            # qkv^T feature-major [128, BT] bf16
            qT = consts.tile([128, BT], bf16, tag="qT")
            kT = consts.tile([128, BT], bf16, tag="kT")
            vT = consts.tile([128, BT], bf16, tag="vT")
            # V in [token-part, kt, head, d|1] layout; col 64 = 1.0 (Z row)
            v_sb = [
                consts.tile([128, QC * KPC, hpc, 65], bf16, tag=f"v{b}",
                            name=f"v{b}") for b in range(Bc)
            ]
            for b in range(Bc):
                nc.gpsimd.memset(v_sb[b][:, :, :, 64:65], 1.0)

            # ------------- emission building blocks -------------
            def qkv_steps(ch):
                """Filler closures: 24 QKV matmuls (+1 eviction each chain)."""
                t_sl = slice(ch * TOKC, (ch + 1) * TOKC)
                steps = []
                for m in range(3):
                    ps_ref = {}
                    for kt in range(KT_C):
                        def step(m=m, kt=kt, ps_ref=ps_ref):
                            if kt == 0:
                                ps_ref[0] = ps_mm.tile([128, TOKC], f32,
                                                       tag="mm", name="qkvps")
                            nc.tensor.matmul(
                                ps_ref[0][:, :],
                                w_sb[:, kt, m * MQ:(m + 1) * MQ],
                                x_sb[:, kt, t_sl],
                                start=(kt == 0), stop=(kt == KT_C - 1))
                            if kt == KT_C - 1:
                                dst = (qT, kT, vT)[m]
                                nc.vector.tensor_scalar_add(
                                    dst[:, t_sl], ps_ref[0][:, :],
                                    b_sb[:, m:m + 1])
                        steps.append(step)
                return steps

            def vt_steps(ch):
                """Filler closures: 4 V^T transposes (+2 DVE evictions each)."""
                b, qc = ch // QC, ch % QC
                steps = []
                for i in range(KPC):
                    kt = qc * KPC + i

                    def step(b=b, kt=kt):
                        pst = ps_s.tile([128, 128], bf16, tag="s", name="vtps")
                        nc.tensor.transpose(
                            pst[:, :],
                            vT[:, b * Tc + kt * 128:b * Tc + (kt + 1) * 128],
                            ident_bf[:, :])
                        for hh in range(hpc):
                            nc.vector.tensor_copy(
                                v_sb[b][:, kt, hh, 0:HD],
                                pst[:, hh * HD:(hh + 1) * HD])
                    steps.append(step)
                return steps

            def proj_steps(b, qc, yT_t, evict_act=False):
                """Filler closures: 8 projection matmuls + evict + DMA.
                evict_act routes the eviction to the scalar engine — used for
                the final chunks where ACT is otherwise idle and the DVE
                backlog was stalling the drain."""
                q_sl = slice(b * Tc + qc * TOKC, b * Tc + (qc + 1) * TOKC)
                steps = []
                for mo in range(MO):
                    def step(mo=mo):
                        pso = ps_mm.tile([128, TOKC], f32, tag="mm",
                                         name="projps")
                        nc.tensor.matmul(
                            pso[:, :], wp_sb[:, mo * 128:(mo + 1) * 128],
                            yT_t[:, :], start=True, stop=True)
                        o_t = opool.tile([128, TOKC], bf16, tag="o")
                        if evict_act and mo % 2 == 0:
                            nc.scalar.activation(out=o_t[:, :], in_=pso[:, :],
                                                 func=AF.Copy)
                        else:
                            nc.vector.tensor_copy(o_t[:, :], pso[:, :])
                        nc.sync.dma_start(
                            out=outT[mo * 128:(mo + 1) * 128, q_sl],
                            in_=o_t[:, :])
                    steps.append(step)
                return steps

            def emit_norm(hh, yT_t, psy, zrow):
                """y^T[hh] = psy[0:64] / broadcast(Z): PE rank-1 broadcast of
                the Z row, then a fast DVE reciprocal on the broadcast (the
                exact DVE `reciprocal` is ~6.5 ns/elem and blocked the DVE
                FIFO for 3.4 us at every chain end)."""
                ps_b = ps_mm.tile([HD, TOKC], f32, tag="mm", name="bcps")
                nc.tensor.matmul(ps_b[:, :], ones_sb[:, :], zrow[:, :],
                                 start=True, stop=True)
                rcb = nbpool.tile([HD, TOKC], f32, tag="rcb")
                nc.vector.reciprocal_approx_fast(rcb[:, :], ps_b[:, :])
                nc.vector.tensor_mul(
                    yT_t[hh * HD:(hh + 1) * HD, :], psy[0:HD, :], rcb[:, :])

            # ------------- woven chunk emission -------------
            pend_av = deque()     # pending AV closures
            norm_pend = deque()   # [countdown_in_av_pops, closure]

            def pop_av():
                pend_av.popleft()()
                for item in norm_pend:
                    item[0] -= 1
                while norm_pend and norm_pend[0][0] <= 0:
                    norm_pend.popleft()[1]()

            def chunk_stream(b, qc, yT_t, filler, filler_late=False):
                n_kt = (qc + 1) * KPC
                qbase = b * Tc + qc * TOKC
                psys = {}

                def emit_S(hh, kt):
                    di = kt - qc * KPC
                    c0 = di * 128 if di >= 0 else 0
                    pss = ps_s.tile([128, TOKC], f32, tag="s")
                    nc.tensor.matmul(
                        pss[:, c0:TOKC],
                        kT[hh * HD:(hh + 1) * HD,
                           b * Tc + kt * 128:b * Tc + (kt + 1) * 128],
                        qT[hh * HD:(hh + 1) * HD, qbase + c0:qbase + TOKC],
                        start=True, stop=True)
                    e_t = spool.tile([128, TOKC], bf16, tag="e")
                    nc.scalar.activation(out=e_t[:, c0:TOKC], in_=pss[:, c0:TOKC],
                                         func=AF.Exp, scale=0.125)
                    if di >= 0:
                        # zero strict upper triangle of the diagonal 128x128
                        # block: keep iff (q - k) = f - p >= 0
                        nc.gpsimd.affine_select(
                            out=e_t[:, c0:c0 + 128], in_=e_t[:, c0:c0 + 128],
                            compare_op=ALU.is_ge, fill=0.0, base=0,
                            pattern=[[1, 128]], channel_multiplier=-1)
                    return e_t, c0

                def mk_av(hh, kt, e_t, c0, psy):
                    def emit():
                        nc.tensor.matmul(
                            psy[:, c0:TOKC], v_sb[b][:, kt, hh, :],
                            e_t[:, c0:TOKC],
                            start=(kt == 0), stop=(kt == n_kt - 1))
                        if kt == n_kt - 1:
                            zrow = npool.tile([1, TOKC], f32r, tag="rc")
                            nc.scalar.activation(out=zrow[:, :],
                                                 in_=psy[64:65, :], func=AF.Copy)
                            norm_pend.append(
                                [2, lambda: emit_norm(hh, yT_t, psy, zrow)])
                    return emit

                n_slots = hpc * n_kt
                slot = 0
                for hh in range(hpc):
                    for kt in range(n_kt):
                        if hh not in psys:
                            psys[hh] = ps_y.tile([65, TOKC], f32, tag="y",
                                                 name=f"psy{hh}")
                        e_t, c0 = emit_S(hh, kt)
                        pend_av.append(mk_av(hh, kt, e_t, c0, psys[hh]))
                        # pace the filler: keep some in reserve for late slots
                        # so tail AVs stay cushioned from their exps; placed
                        # between S and AV so the AV weight-load can prefetch
                        # behind the filler matmul
                        if filler and (len(filler) >= n_slots - slot
                                       or (slot % 2 == 0 and not filler_late)):
                            filler.popleft()()
                        if len(pend_av) > DEPTH:
                            pop_av()
                        slot += 1
                # drain: alternate remaining filler with pending AVs/norms
                while filler or pend_av or norm_pend:
                    if filler:
                        filler.popleft()()
                    if pend_av:
                        pop_av()
                    elif not filler:
                        while norm_pend:
                            norm_pend.popleft()[1]()

            # ---------------- main schedule ----------------
            # prologue: QKV + V^T for chunk 0, unwoven
            for step in qkv_steps(0):
                step()
            for step in vt_steps(0):
                step()

            pending_proj = None   # (b, qc, yT_t) of previous chunk
            for g in range(NCH):
                b, qc = g // QC, g % QC
                filler = deque()
                if g + 1 < NCH:
                    filler.extend(qkv_steps(g + 1))
                    filler.extend(vt_steps(g + 1))
                if pending_proj is not None:
                    filler.extend(proj_steps(*pending_proj,
                                             evict_act=(g == NCH - 1)))
                yT_t = ypool.tile([128, TOKC], bf16, tag="yT")
                chunk_stream(b, qc, yT_t, filler, filler_late=(g == NCH - 1))
                pending_proj = (b, qc, yT_t)
            # epilogue: projection of the last chunk
            for step in proj_steps(*pending_proj, evict_act=True):
                step()

    nc.finalize()
    return nc


def prep_inputs(cfg, x, W_attn, b_attn, W_proj, b_proj):
    """Host-side sharding: returns per-core input dicts."""
    import ml_dtypes

    Bc, Tc, Cc, hpc = cfg["B"], cfg["T"], cfg["C"], cfg["HPC"]
    n_cores = (Cc // HD) // hpc
    BT = Bc * Tc
    MQ = hpc * HD
    bf16 = ml_dtypes.bfloat16

    x = np.asarray(x, dtype=np.float32)
    xT = np.ascontiguousarray(x.reshape(BT, Cc).T.astype(bf16))

    in_maps = []
    for c in range(n_cores):
        r0 = c * MQ
        rows = []
        for g in range(3):
            rows.append(np.arange(g * Cc + r0, g * Cc + r0 + MQ))
        rows = np.concatenate(rows)
        w_slice = np.asarray(W_attn, dtype=np.float32)[rows, :]   # [384, C]
        wqkvT = np.ascontiguousarray(w_slice.T.astype(bf16))      # [C, 384]
        bq = np.ascontiguousarray(
            np.asarray(b_attn, dtype=np.float32)[rows].reshape(MQ * 3, 1))
        wpT = np.ascontiguousarray(
            np.asarray(W_proj, dtype=np.float32)[:, r0:r0 + MQ].T.astype(bf16))
        in_maps.append({
            "xT": xT,
            "wqkvT": wqkvT,
            "bqkv": bq.astype(np.float32),
            "wpT": wpT,
        })
    return in_maps


def combine(cfg, results, b_proj):
    Bc, Tc, Cc = cfg["B"], cfg["T"], cfg["C"]
    acc = results[0]["outT"].astype(np.float32)
    for r in results[1:]:
        acc = acc + r["outT"].astype(np.float32)
    out = acc.T + np.asarray(b_proj, dtype=np.float32)[None, :]
    return np.ascontiguousarray(out.reshape(Bc, Tc, Cc).astype(np.float32))


_NC_CACHE = {}


def kernel(x, W_attn, b_attn, W_proj, b_proj):
    from concourse.bass_utils import run_bass_kernel_spmd

    cfg = _cfg_full()
    key = "full"
    if key not in _NC_CACHE:
        _NC_CACHE[key] = build_nc(cfg)
    nc = _NC_CACHE[key]
    in_maps = prep_inputs(cfg, np.asarray(x), np.asarray(W_attn),
                          np.asarray(b_attn), np.asarray(W_proj),
                          np.asarray(b_proj))
    res = run_bass_kernel_spmd(nc, in_maps, list(range(N_CORES)))
    return combine(cfg, res.results, np.asarray(b_proj, dtype=np.float32))
